# revision 1
# baseline (speedup 1.0000x reference)
"""TRN2 Bass kernel: 4096x4096 fp32 'valid' cross-correlation with a 15x15
kernel (+ scalar bias), sharded row-wise across 8 NeuronCores.

Formulation (per core, per output row-tile of M<=114 rows):
  out[i, j] = sum_dj sum_di w[di, dj] * x[i + di, j + dj]
For each kernel column dj, the contraction over di is a banded-Toeplitz
matmul over input rows: stationary lhsT[k, m] = w[k - m, dj] (15-diagonal
band, K = M + 14 input rows), moving rhs = x rows with a free-dim column
offset of dj. The 15 dj-matmuls accumulate in one PSUM bank. Matmuls run
in float32r (TF32): 1 col/cycle on the PE at ~1e-4 relative error.

Each core gets 512 padded output rows (input slice of 526 rows including
the 14-row halo); the host pads x to 4110 rows and drops the 14 garbage
output rows at the end.
"""

import os
import sys

for _p in ("/opt/trn_rl_repo",):
    if os.path.isdir(_p) and _p not in sys.path:
        sys.path.insert(0, _p)

import json

import numpy as np

import concourse.bass as bass
import concourse.tile as tile
from concourse import mybir
from concourse.bass_utils import run_bass_kernel_spmd

# ---------------------------------------------------------------------------
# Workaround: the installed walrus_driver rejects instructions carrying more
# than one sync wait ("Too many sync wait commands"). TileContext's kernel-tail
# drain carries one wait per outstanding semaphore. Splitting each extra wait
# into its own single-wait EventSemaphore on the same engine right before the
# original instruction is semantically identical (same-engine program order;
# semaphores are monotone).
# ---------------------------------------------------------------------------
_orig_to_json_bytes = bass.Bass.to_json_bytes


def _split_multi_waits(bir: dict) -> dict:
    n = 0
    for fn in bir.get("functions", []):
        for bb in fn.get("blocks", []):
            insts = bb.get("instructions")
            if not insts:
                continue
            out = []
            for inst in insts:
                si = inst.get("sync_info") or {}
                waits = si.get("on_wait") or []
                if len(waits) > 1:
                    for w in waits[:-1]:
                        n += 1
                        out.append(
                            {
                                "debug": inst.get("debug", 0),
                                "engine": inst["engine"],
                                "ins": [],
                                "name": f"{inst['name']}-waitsplit{n}",
                                "opcode": "EventSemaphore",
                                "outs": [],
                                "sync_info": {"on_update": [], "on_wait": [w]},
                            }
                        )
                    si["on_wait"] = [waits[-1]]
                out.append(inst)
            bb["instructions"] = out
    return bir


def _patched_to_json_bytes(self, *args, **kwargs):
    raw = _orig_to_json_bytes(self, *args, **kwargs)
    return json.dumps(_split_multi_waits(json.loads(raw))).encode()


bass.Bass.to_json_bytes = _patched_to_json_bytes

# ---------------------------------------------------------------------------

H = W = 4096
KS = 15
OUT_H = H - KS + 1  # 4082
OUT_W = W - KS + 1  # 4082
N_CORES = 8
ROWS_PER_CORE = 512  # padded output rows per core (8 * 512 = 4096 >= 4082)
IN_ROWS = ROWS_PER_CORE + KS - 1  # 526

# Output row-tiles per core: M <= 114 so the band (M + 14) fits in K <= 128.
M_TILES = [(0, 114), (114, 114), (228, 114), (342, 114), (456, 56)]
# Output column tiles: N <= 512 (one fp32 PSUM bank).
N_TILES = [(c, min(512, OUT_W - c)) for c in range(0, OUT_W, 512)]

F32R = mybir.dt.float32r
F32 = mybir.dt.float32


def build_program(bias_val: float, repeat: int = 1) -> bass.Bass:
    nc = bass.Bass()
    x_d = nc.dram_tensor("x", [IN_ROWS, W], F32R, kind="ExternalInput")
    t_d = nc.dram_tensor("t", [128, KS, 128], F32R, kind="ExternalInput")
    y_d = nc.dram_tensor("y", [ROWS_PER_CORE, OUT_W], F32, kind="ExternalOutput")

    with tile.TileContext(nc) as tc:
        with (
            tc.tile_pool(name="tconst", bufs=1) as tpool,
            tc.tile_pool(name="xin", bufs=2) as xpool,
            tc.tile_pool(name="yout", bufs=2) as ypool,
            tc.tile_pool(name="acc", bufs=4, space="PSUM") as psum,
        ):
            t_s = tpool.tile([128, KS, 128], F32R)
            nc.sync.dma_start(t_s[:], t_d[:])
            for _ in range(repeat):
                for m0, m in M_TILES:
                    kp = m + KS - 1
                    x_s = xpool.tile([128, W], F32R)
                    nc.sync.dma_start(x_s[0:kp, :], x_d[m0 : m0 + kp, :])
                    y_s = ypool.tile([128, OUT_W], F32)
                    for c0, n in N_TILES:
                        acc = psum.tile([128, 512], F32)
                        for dj in range(KS):
                            nc.tensor.matmul(
                                acc[0:m, 0:n],
                                t_s[0:kp, dj, 0:m],
                                x_s[0:kp, c0 + dj : c0 + dj + n],
                                start=(dj == 0),
                                stop=(dj == KS - 1),
                            )
                        nc.vector.tensor_scalar_add(
                            y_s[0:m, c0 : c0 + n], acc[0:m, 0:n], bias_val
                        )
                    nc.sync.dma_start(y_d[m0 : m0 + m, :], y_s[0:m, :])
    return nc


def _toeplitz(weight: np.ndarray) -> np.ndarray:
    """T[k, dj, m] = weight[k - m, dj] for 0 <= k - m < 15, else 0."""
    t = np.zeros((128, KS, 128), dtype=np.float32)
    k = np.arange(128)[:, None]
    m = np.arange(128)[None, :]
    d = k - m  # [128, 128]
    mask = (d >= 0) & (d < KS)
    for dj in range(KS):
        col = np.zeros((128, 128), dtype=np.float32)
        col[mask] = weight[d[mask], dj]
        t[:, dj, :] = col
    return t


def _prepare_inputs(x: np.ndarray, weight: np.ndarray):
    x_pad = np.zeros((N_CORES * ROWS_PER_CORE + KS - 1, W), dtype=np.float32)
    x_pad[:H] = x
    t = _toeplitz(weight)
    in_maps = []
    for c in range(N_CORES):
        r0 = c * ROWS_PER_CORE
        in_maps.append({"x": np.ascontiguousarray(x_pad[r0 : r0 + IN_ROWS]), "t": t})
    return in_maps


def run(x: np.ndarray, weight: np.ndarray, bias: np.ndarray, repeat: int = 1):
    nc = build_program(float(bias[0]), repeat=repeat)
    in_maps = _prepare_inputs(x, weight)
    res = run_bass_kernel_spmd(nc, in_maps, list(range(N_CORES)))
    full = np.concatenate([res.results[c]["y"] for c in range(N_CORES)], axis=0)
    return np.ascontiguousarray(full[:OUT_H]).astype(np.float32)


def kernel(x: np.ndarray, weight: np.ndarray, bias: np.ndarray) -> np.ndarray:
    return run(x, weight, bias, repeat=1)


# revision 16
# speedup vs baseline: 481.1239x; 481.1239x over previous
"""TRN2 Bass kernel: 4096x4096 fp32 'valid' cross-correlation with a 15x15
kernel (+ scalar bias), sharded row-wise across 8 NeuronCores.

Formulation (per core, per output row-tile of M<=114 rows):
  out[i, j] = sum_dj sum_di w[di, dj] * x[i + di, j + dj]
For each kernel column dj, the contraction over di is a banded-Toeplitz
matmul over input rows: stationary lhsT[k, m] = w[k - m, dj] (15-diagonal
band, K = M + 14 input rows), moving rhs = x rows with a free-dim column
offset of dj. The 15 dj-matmuls accumulate in one PSUM bank. Matmuls run
in float32r (TF32): 1 col/cycle on the PE at ~1e-4 relative error.

Each core gets 512 padded output rows (input slice of 526 rows including
the 14-row halo); the host pads x to 4110 rows and drops the 14 garbage
output rows at the end.
"""

import os
import sys

for _p in ("/opt/trn_rl_repo",):
    if os.path.isdir(_p) and _p not in sys.path:
        sys.path.insert(0, _p)

import json

import numpy as np

import concourse.bass as bass
import concourse.tile as tile
from concourse import mybir
from concourse.bass_utils import run_bass_kernel_spmd

# ---------------------------------------------------------------------------
# Workaround: the installed walrus_driver rejects instructions carrying more
# than one sync wait ("Too many sync wait commands"). TileContext's kernel-tail
# drain carries one wait per outstanding semaphore. Splitting each extra wait
# into its own single-wait EventSemaphore on the same engine right before the
# original instruction is semantically identical (same-engine program order;
# semaphores are monotone).
# ---------------------------------------------------------------------------
_orig_to_json_bytes = bass.Bass.to_json_bytes


def _split_multi_waits(bir: dict) -> dict:
    n = 0
    for fn in bir.get("functions", []):
        for bb in fn.get("blocks", []):
            insts = bb.get("instructions")
            if not insts:
                continue
            out = []
            for inst in insts:
                si = inst.get("sync_info") or {}
                waits = si.get("on_wait") or []
                if len(waits) > 1:
                    for w in waits[:-1]:
                        n += 1
                        out.append(
                            {
                                "debug": inst.get("debug", 0),
                                "engine": inst["engine"],
                                "ins": [],
                                "name": f"{inst['name']}-waitsplit{n}",
                                "opcode": "EventSemaphore",
                                "outs": [],
                                "sync_info": {"on_update": [], "on_wait": [w]},
                            }
                        )
                    si["on_wait"] = [waits[-1]]
                out.append(inst)
            bb["instructions"] = out
    return bir


def _patched_to_json_bytes(self, *args, **kwargs):
    raw = _orig_to_json_bytes(self, *args, **kwargs)
    return json.dumps(_split_multi_waits(json.loads(raw))).encode()


bass.Bass.to_json_bytes = _patched_to_json_bytes

# ---------------------------------------------------------------------------

H = W = 4096
KS = 15
OUT_H = H - KS + 1  # 4082
OUT_W = W - KS + 1  # 4082
N_CORES = 8
ROWS_PER_CORE = 512  # padded output rows per core (8 * 512 = 4096 >= 4082)
IN_ROWS = ROWS_PER_CORE + KS - 1  # 526

# Output row-tiles per core: M <= 114 so the band (M + 14) fits in K <= 128.
M_TILES = [(0, 114), (114, 114), (228, 114), (342, 114), (456, 56)]
# Corner scheme: 4 tiles of 128 rows; the 14 seam rows per tile are completed
# by two extra matmuls contracting (dj, k') pairs over replicated shifted
# copies of the 14 rows below the tile's K-block.
M_TILES_CORNER = [(0, 128), (128, 128), (256, 128), (384, 128)]
CORNER_SPLIT = 9  # dj 0..8 -> corner MM a (K=126), dj 9..14 -> MM b (K=84)
# Output column tiles: N <= 512 (one fp32 PSUM bank).
N_TILES = [(c, min(512, OUT_W - c)) for c in range(0, OUT_W, 512)]

F32R = mybir.dt.float32r
F32 = mybir.dt.float32


def build_program(
    bias_val: float,
    repeat: int = 1,
    loop_repeat: int = 1,
    loop_order: str = "c_dj",
    evacuate: bool = True,
    psum_bufs: int = 4,
    same_stationary: bool = False,
    pure_mm: bool = False,
    x_bufs: int = 2,
    y_per_ctile: bool = False,
) -> bass.Bass:
    nc = bass.Bass()
    x_d = nc.dram_tensor("x", [IN_ROWS, W], F32R, kind="ExternalInput")
    t_d = nc.dram_tensor("t", [128, KS, 128], F32R, kind="ExternalInput")
    t2_d = nc.dram_tensor("t2", [14 * KS, 128], F32R, kind="ExternalInput")
    y_d = nc.dram_tensor("y", [ROWS_PER_CORE, OUT_W], F32, kind="ExternalOutput")

    with tile.TileContext(nc) as tc:
        with (
            tc.tile_pool(name="tconst", bufs=1) as tpool,
            tc.tile_pool(name="xin", bufs=x_bufs) as xpool,
            tc.tile_pool(name="rrep", bufs=2) as rpool,
            tc.tile_pool(name="yout", bufs=2) as ypool,
            tc.tile_pool(name="acc", bufs=psum_bufs, space="PSUM") as psum,
        ):
            t_s = tpool.tile([128, KS, 128], F32R)
            nc.sync.dma_start(t_s[:], t_d[:])
            ka = 14 * CORNER_SPLIT  # 126
            kb = 14 * (KS - CORNER_SPLIT)  # 84
            if loop_order == "corner":
                t2a_s = tpool.tile([ka, 128], F32R)
                t2b_s = tpool.tile([kb, 128], F32R)
                nc.sync.dma_start(t2a_s[:], t2_d[0:ka, :])
                nc.sync.dma_start(t2b_s[:], t2_d[ka : ka + kb, :])

            def mtile_c_dj(m0, m, kp, x_s, y_s):
                for c0, n in N_TILES:
                    acc = psum.tile([128, 512], F32, tag="acc")
                    for dj in range(KS):
                        nc.tensor.matmul(
                            acc[0:m, 0:n],
                            t_s[0:kp, 0 if same_stationary else dj, 0:m],
                            x_s[0:kp, c0 + dj : c0 + dj + n],
                            start=(dj == 0),
                            stop=(dj == KS - 1),
                        )
                    if evacuate:
                        nc.vector.tensor_scalar_add(
                            y_s[0:m, c0 : c0 + n], acc[0:m, 0:n], bias_val
                        )
                        if y_per_ctile:
                            nc.sync.dma_start(
                                y_d[m0 : m0 + m, c0 : c0 + n],
                                y_s[0:m, c0 : c0 + n],
                            )

            def mtile_dj_c(m0, m, kp, x_s, y_s):
                accs = [
                    psum.tile([128, 512], F32, tag=f"acc{i}", name=f"acc{i}")
                    for i in range(len(N_TILES))
                ]
                for dj in range(KS):
                    for ci, (c0, n) in enumerate(N_TILES):
                        nc.tensor.matmul(
                            accs[ci][0:m, 0:n],
                            t_s[0:kp, dj, 0:m],
                            x_s[0:kp, c0 + dj : c0 + dj + n],
                            start=(dj == 0),
                            stop=(dj == KS - 1),
                        )
                if evacuate:
                    for ci, (c0, n) in enumerate(N_TILES):
                        nc.vector.tensor_scalar_add(
                            y_s[0:m, c0 : c0 + n], accs[ci][0:m, 0:n], bias_val
                        )

            def mtile_corner(m0, x_s, ra, rb, y_s):
                for c0, n in N_TILES:
                    acc = psum.tile([128, 512], F32, tag="acc")
                    for dj in range(KS):
                        nc.tensor.matmul(
                            acc[:, 0:n],
                            t_s[:, dj, :],
                            x_s[:, c0 + dj : c0 + dj + n],
                            start=(dj == 0),
                            stop=False,
                        )
                    nc.tensor.matmul(
                        acc[:, 0:n],
                        t2a_s[:],
                        ra[0:ka, c0 : c0 + n],
                        start=False,
                        stop=False,
                    )
                    nc.tensor.matmul(
                        acc[:, 0:n],
                        t2b_s[:],
                        rb[0:kb, c0 : c0 + n],
                        start=False,
                        stop=True,
                    )
                    if evacuate:
                        nc.vector.tensor_scalar_add(
                            y_s[:, c0 : c0 + n], acc[:, 0:n], bias_val
                        )

            def body_corner():
                for _ in range(repeat):
                    for m0, m in M_TILES_CORNER:
                        x_s = xpool.tile([128, W], F32R, tag="xs")
                        nc.sync.dma_start(x_s[:], x_d[m0 : m0 + 128, :])
                        ra = rpool.tile([128, OUT_W], F32R, tag="ra")
                        rb = rpool.tile([128, OUT_W], F32R, tag="rb")
                        for dj in range(KS):
                            dst = ra if dj < CORNER_SPLIT else rb
                            p0 = 14 * (dj if dj < CORNER_SPLIT else dj - CORNER_SPLIT)
                            nc.sync.dma_start(
                                dst[p0 : p0 + 14, :],
                                x_d[m0 + 128 : m0 + 142, dj : dj + OUT_W],
                            )
                        y_s = ypool.tile([128, OUT_W], F32, tag="ys")
                        mtile_corner(m0, x_s, ra, rb, y_s)
                        if evacuate:
                            nc.sync.dma_start(y_d[m0 : m0 + 128, :], y_s[:])

            def body():
                if loop_order == "corner":
                    body_corner()
                    return
                for _ in range(repeat):
                    for m0, m in M_TILES:
                        kp = m + KS - 1
                        x_s = xpool.tile([128, W], F32R, tag="xs")
                        nc.sync.dma_start(x_s[0:kp, :], x_d[m0 : m0 + kp, :])
                        y_s = ypool.tile([128, OUT_W], F32, tag="ys")
                        if loop_order == "c_dj":
                            mtile_c_dj(m0, m, kp, x_s, y_s)
                        else:
                            mtile_dj_c(m0, m, kp, x_s, y_s)
                        if evacuate and not (loop_order == "c_dj" and y_per_ctile):
                            nc.sync.dma_start(y_d[m0 : m0 + m, :], y_s[0:m, :])

            def body_pure_mm():
                x_s = xpool.tile([128, W], F32R, tag="xs")
                nc.sync.dma_start(x_s[:], x_d[0:128, :])

                def inner():
                    for _ in range(repeat):
                        for m0, m in M_TILES:
                            kp = m + KS - 1
                            for c0, n in N_TILES:
                                acc = psum.tile([128, 512], F32, tag="acc")
                                for dj in range(KS):
                                    nc.tensor.matmul(
                                        acc[0:m, 0:n],
                                        t_s[0:kp, dj, 0:m],
                                        x_s[0:kp, c0 + dj : c0 + dj + n],
                                        start=(dj == 0),
                                        stop=(dj == KS - 1),
                                    )

                if loop_repeat > 1:
                    with tc.For_i(0, loop_repeat, 1):
                        inner()
                else:
                    inner()

            if pure_mm:
                body_pure_mm()
            elif loop_repeat > 1:
                with tc.For_i(0, loop_repeat, 1):
                    body()
            else:
                body()
    return nc


def _toeplitz(weight: np.ndarray) -> np.ndarray:
    """T[k, dj, m] = weight[k - m, dj] for 0 <= k - m < 15, else 0."""
    t = np.zeros((128, KS, 128), dtype=np.float32)
    k = np.arange(128)[:, None]
    m = np.arange(128)[None, :]
    d = k - m  # [128, 128]
    mask = (d >= 0) & (d < KS)
    for dj in range(KS):
        col = np.zeros((128, 128), dtype=np.float32)
        col[mask] = weight[d[mask], dj]
        t[:, dj, :] = col
    return t


def _toeplitz_corner(weight: np.ndarray) -> np.ndarray:
    """T2[14*dj + k', m] = weight[128 + k' - m, dj] for m in [114+k', 127]."""
    t2 = np.zeros((14 * KS, 128), dtype=np.float32)
    for dj in range(KS):
        for k_ in range(14):
            m = np.arange(114 + k_, 128)
            t2[14 * dj + k_, m] = weight[128 + k_ - m, dj]
    return t2


def _prepare_inputs(x: np.ndarray, weight: np.ndarray):
    x_pad = np.zeros((N_CORES * ROWS_PER_CORE + KS - 1, W), dtype=np.float32)
    x_pad[:H] = x
    t = _toeplitz(weight)
    t2 = _toeplitz_corner(weight)
    in_maps = []
    for c in range(N_CORES):
        r0 = c * ROWS_PER_CORE
        in_maps.append(
            {"x": np.ascontiguousarray(x_pad[r0 : r0 + IN_ROWS]), "t": t, "t2": t2}
        )
    return in_maps


DEFAULT_BUILD = {"loop_order": "c_dj"}


def run(x: np.ndarray, weight: np.ndarray, bias: np.ndarray, repeat: int = 1, **kw):
    bkw = {**DEFAULT_BUILD, **kw}
    nc = build_program(float(bias[0]), repeat=repeat, **bkw)
    in_maps = _prepare_inputs(x, weight)
    res = run_bass_kernel_spmd(nc, in_maps, list(range(N_CORES)))
    full = np.concatenate([res.results[c]["y"] for c in range(N_CORES)], axis=0)
    return np.ascontiguousarray(full[:OUT_H]).astype(np.float32)


def kernel(x: np.ndarray, weight: np.ndarray, bias: np.ndarray) -> np.ndarray:
    return run(x, weight, bias, repeat=1)


# revision 18
# speedup vs baseline: 544.8711x; 1.1325x over previous
"""TRN2 Bass kernel: 4096x4096 fp32 'valid' cross-correlation with a 15x15
kernel (+ scalar bias), sharded row-wise across 8 NeuronCores.

Formulation (per core, per output row-tile of M<=114 rows):
  out[i, j] = sum_dj sum_di w[di, dj] * x[i + di, j + dj]
For each kernel column dj, the contraction over di is a banded-Toeplitz
matmul over input rows: stationary lhsT[k, m] = w[k - m, dj] (15-diagonal
band, K = M + 14 input rows), moving rhs = x rows with a free-dim column
offset of dj. The 15 dj-matmuls accumulate in one PSUM bank. Matmuls run
in float32r (TF32): 1 col/cycle on the PE at ~1e-4 relative error.

Each core gets 512 padded output rows (input slice of 526 rows including
the 14-row halo); the host pads x to 4110 rows and drops the 14 garbage
output rows at the end.
"""

import os
import sys

for _p in ("/opt/trn_rl_repo",):
    if os.path.isdir(_p) and _p not in sys.path:
        sys.path.insert(0, _p)

import json

import numpy as np

import concourse.bass as bass
import concourse.tile as tile
from concourse import mybir
from concourse.bass_utils import run_bass_kernel_spmd

# ---------------------------------------------------------------------------
# Workaround: the installed walrus_driver rejects instructions carrying more
# than one sync wait ("Too many sync wait commands"). TileContext's kernel-tail
# drain carries one wait per outstanding semaphore. Splitting each extra wait
# into its own single-wait EventSemaphore on the same engine right before the
# original instruction is semantically identical (same-engine program order;
# semaphores are monotone).
# ---------------------------------------------------------------------------
_orig_to_json_bytes = bass.Bass.to_json_bytes


def _split_multi_waits(bir: dict) -> dict:
    n = 0
    for fn in bir.get("functions", []):
        for bb in fn.get("blocks", []):
            insts = bb.get("instructions")
            if not insts:
                continue
            out = []
            for inst in insts:
                si = inst.get("sync_info") or {}
                waits = si.get("on_wait") or []
                if len(waits) > 1:
                    for w in waits[:-1]:
                        n += 1
                        out.append(
                            {
                                "debug": inst.get("debug", 0),
                                "engine": inst["engine"],
                                "ins": [],
                                "name": f"{inst['name']}-waitsplit{n}",
                                "opcode": "EventSemaphore",
                                "outs": [],
                                "sync_info": {"on_update": [], "on_wait": [w]},
                            }
                        )
                    si["on_wait"] = [waits[-1]]
                out.append(inst)
            bb["instructions"] = out
    return bir


def _patched_to_json_bytes(self, *args, **kwargs):
    raw = _orig_to_json_bytes(self, *args, **kwargs)
    return json.dumps(_split_multi_waits(json.loads(raw))).encode()


bass.Bass.to_json_bytes = _patched_to_json_bytes

# ---------------------------------------------------------------------------

H = W = 4096
KS = 15
OUT_H = H - KS + 1  # 4082
OUT_W = W - KS + 1  # 4082
N_CORES = 8
ROWS_PER_CORE = 512  # padded output rows per core (8 * 512 = 4096 >= 4082)
IN_ROWS = ROWS_PER_CORE + KS - 1  # 526

# Output row-tiles per core: M <= 114 so the band (M + 14) fits in K <= 128.
M_TILES = [(0, 114), (114, 114), (228, 114), (342, 114), (456, 56)]
# Corner scheme: 4 tiles of 128 rows; the 14 seam rows per tile are completed
# by two extra matmuls contracting (dj, k') pairs over replicated shifted
# copies of the 14 rows below the tile's K-block.
M_TILES_CORNER = [(0, 128), (128, 128), (256, 128), (384, 128)]
CORNER_SPLIT = 9  # dj 0..8 -> corner MM a (K=126), dj 9..14 -> MM b (K=84)
# Output column tiles: N <= 512 (one fp32 PSUM bank).
N_TILES = [(c, min(512, OUT_W - c)) for c in range(0, OUT_W, 512)]

F32R = mybir.dt.float32r
F32 = mybir.dt.float32


def build_program(
    bias_val: float,
    repeat: int = 1,
    loop_repeat: int = 1,
    loop_order: str = "c_dj",
    evacuate: bool = True,
    psum_bufs: int = 4,
    same_stationary: bool = False,
    pure_mm: bool = False,
    x_bufs: int = 2,
    y_per_ctile: bool = False,
    split_dma: int = 1,
    evac_any: bool = False,
) -> bass.Bass:
    nc = bass.Bass()
    x_d = nc.dram_tensor("x", [IN_ROWS, W], F32R, kind="ExternalInput")
    t_d = nc.dram_tensor("t", [128, KS, 128], F32R, kind="ExternalInput")
    t2_d = nc.dram_tensor("t2", [14 * KS, 128], F32R, kind="ExternalInput")
    y_d = nc.dram_tensor("y", [ROWS_PER_CORE, OUT_W], F32, kind="ExternalOutput")

    with tile.TileContext(nc) as tc:
        with (
            tc.tile_pool(name="tconst", bufs=1) as tpool,
            tc.tile_pool(name="xin", bufs=x_bufs) as xpool,
            tc.tile_pool(name="rrep", bufs=2) as rpool,
            tc.tile_pool(name="yout", bufs=2) as ypool,
            tc.tile_pool(name="acc", bufs=psum_bufs, space="PSUM") as psum,
        ):
            t_s = tpool.tile([128, KS, 128], F32R)
            nc.sync.dma_start(t_s[:], t_d[:])
            ka = 14 * CORNER_SPLIT  # 126
            kb = 14 * (KS - CORNER_SPLIT)  # 84
            if loop_order == "corner":
                t2a_s = tpool.tile([ka, 128], F32R)
                t2b_s = tpool.tile([kb, 128], F32R)
                nc.sync.dma_start(t2a_s[:], t2_d[0:ka, :])
                nc.sync.dma_start(t2b_s[:], t2_d[ka : ka + kb, :])

            def mtile_c_dj(m0, m, kp, x_s, y_s):
                for c0, n in N_TILES:
                    acc = psum.tile([128, 512], F32, tag="acc")
                    for dj in range(KS):
                        nc.tensor.matmul(
                            acc[0:m, 0:n],
                            t_s[0:kp, 0 if same_stationary else dj, 0:m],
                            x_s[0:kp, c0 + dj : c0 + dj + n],
                            start=(dj == 0),
                            stop=(dj == KS - 1),
                        )
                    if evacuate:
                        eng = nc.any if evac_any else nc.vector
                        eng.tensor_scalar_add(
                            y_s[0:m, c0 : c0 + n], acc[0:m, 0:n], bias_val
                        )
                        if y_per_ctile:
                            nc.sync.dma_start(
                                y_d[m0 : m0 + m, c0 : c0 + n],
                                y_s[0:m, c0 : c0 + n],
                            )

            def mtile_dj_c(m0, m, kp, x_s, y_s):
                accs = [
                    psum.tile([128, 512], F32, tag=f"acc{i}", name=f"acc{i}")
                    for i in range(len(N_TILES))
                ]
                for dj in range(KS):
                    for ci, (c0, n) in enumerate(N_TILES):
                        nc.tensor.matmul(
                            accs[ci][0:m, 0:n],
                            t_s[0:kp, dj, 0:m],
                            x_s[0:kp, c0 + dj : c0 + dj + n],
                            start=(dj == 0),
                            stop=(dj == KS - 1),
                        )
                if evacuate:
                    for ci, (c0, n) in enumerate(N_TILES):
                        nc.vector.tensor_scalar_add(
                            y_s[0:m, c0 : c0 + n], accs[ci][0:m, 0:n], bias_val
                        )

            def mtile_corner(m0, x_s, ra, rb, y_s):
                for c0, n in N_TILES:
                    acc = psum.tile([128, 512], F32, tag="acc")
                    for dj in range(KS):
                        nc.tensor.matmul(
                            acc[:, 0:n],
                            t_s[:, dj, :],
                            x_s[:, c0 + dj : c0 + dj + n],
                            start=(dj == 0),
                            stop=False,
                        )
                    nc.tensor.matmul(
                        acc[:, 0:n],
                        t2a_s[:],
                        ra[0:ka, c0 : c0 + n],
                        start=False,
                        stop=False,
                    )
                    nc.tensor.matmul(
                        acc[:, 0:n],
                        t2b_s[:],
                        rb[0:kb, c0 : c0 + n],
                        start=False,
                        stop=True,
                    )
                    if evacuate:
                        nc.vector.tensor_scalar_add(
                            y_s[:, c0 : c0 + n], acc[:, 0:n], bias_val
                        )

            def body_corner():
                for _ in range(repeat):
                    for m0, m in M_TILES_CORNER:
                        x_s = xpool.tile([128, W], F32R, tag="xs")
                        nc.sync.dma_start(x_s[:], x_d[m0 : m0 + 128, :])
                        ra = rpool.tile([128, OUT_W], F32R, tag="ra")
                        rb = rpool.tile([128, OUT_W], F32R, tag="rb")
                        for dj in range(KS):
                            dst = ra if dj < CORNER_SPLIT else rb
                            p0 = 14 * (dj if dj < CORNER_SPLIT else dj - CORNER_SPLIT)
                            nc.sync.dma_start(
                                dst[p0 : p0 + 14, :],
                                x_d[m0 + 128 : m0 + 142, dj : dj + OUT_W],
                            )
                        y_s = ypool.tile([128, OUT_W], F32, tag="ys")
                        mtile_corner(m0, x_s, ra, rb, y_s)
                        if evacuate:
                            nc.sync.dma_start(y_d[m0 : m0 + 128, :], y_s[:])

            def body():
                if loop_order == "corner":
                    body_corner()
                    return
                for _ in range(repeat):
                    for m0, m in M_TILES:
                        kp = m + KS - 1
                        x_s = xpool.tile([128, W], F32R, tag="xs")
                        if split_dma > 1:
                            step = (kp + split_dma - 1) // split_dma
                            for p in range(0, kp, step):
                                pe = min(p + step, kp)
                                nc.sync.dma_start(
                                    x_s[p:pe, :], x_d[m0 + p : m0 + pe, :]
                                )
                        else:
                            nc.sync.dma_start(x_s[0:kp, :], x_d[m0 : m0 + kp, :])
                        y_s = ypool.tile([128, OUT_W], F32, tag="ys")
                        if loop_order == "c_dj":
                            mtile_c_dj(m0, m, kp, x_s, y_s)
                        else:
                            mtile_dj_c(m0, m, kp, x_s, y_s)
                        if evacuate and not (loop_order == "c_dj" and y_per_ctile):
                            if split_dma > 1:
                                cstep = (OUT_W + split_dma - 1) // split_dma
                                for c in range(0, OUT_W, cstep):
                                    ce = min(c + cstep, OUT_W)
                                    nc.sync.dma_start(
                                        y_d[m0 : m0 + m, c:ce], y_s[0:m, c:ce]
                                    )
                            else:
                                nc.sync.dma_start(y_d[m0 : m0 + m, :], y_s[0:m, :])

            def body_pure_mm():
                x_s = xpool.tile([128, W], F32R, tag="xs")
                nc.sync.dma_start(x_s[:], x_d[0:128, :])

                def inner():
                    for _ in range(repeat):
                        for m0, m in M_TILES:
                            kp = m + KS - 1
                            for c0, n in N_TILES:
                                acc = psum.tile([128, 512], F32, tag="acc")
                                for dj in range(KS):
                                    nc.tensor.matmul(
                                        acc[0:m, 0:n],
                                        t_s[0:kp, dj, 0:m],
                                        x_s[0:kp, c0 + dj : c0 + dj + n],
                                        start=(dj == 0),
                                        stop=(dj == KS - 1),
                                    )

                if loop_repeat > 1:
                    with tc.For_i(0, loop_repeat, 1):
                        inner()
                else:
                    inner()

            if pure_mm:
                body_pure_mm()
            elif loop_repeat > 1:
                with tc.For_i(0, loop_repeat, 1):
                    body()
            else:
                body()
    return nc


def _toeplitz(weight: np.ndarray) -> np.ndarray:
    """T[k, dj, m] = weight[k - m, dj] for 0 <= k - m < 15, else 0."""
    t = np.zeros((128, KS, 128), dtype=np.float32)
    k = np.arange(128)[:, None]
    m = np.arange(128)[None, :]
    d = k - m  # [128, 128]
    mask = (d >= 0) & (d < KS)
    for dj in range(KS):
        col = np.zeros((128, 128), dtype=np.float32)
        col[mask] = weight[d[mask], dj]
        t[:, dj, :] = col
    return t


def _toeplitz_corner(weight: np.ndarray) -> np.ndarray:
    """T2[14*dj + k', m] = weight[128 + k' - m, dj] for m in [114+k', 127]."""
    t2 = np.zeros((14 * KS, 128), dtype=np.float32)
    for dj in range(KS):
        for k_ in range(14):
            m = np.arange(114 + k_, 128)
            t2[14 * dj + k_, m] = weight[128 + k_ - m, dj]
    return t2


def _prepare_inputs(x: np.ndarray, weight: np.ndarray):
    x_pad = np.zeros((N_CORES * ROWS_PER_CORE + KS - 1, W), dtype=np.float32)
    x_pad[:H] = x
    t = _toeplitz(weight)
    t2 = _toeplitz_corner(weight)
    in_maps = []
    for c in range(N_CORES):
        r0 = c * ROWS_PER_CORE
        in_maps.append(
            {"x": np.ascontiguousarray(x_pad[r0 : r0 + IN_ROWS]), "t": t, "t2": t2}
        )
    return in_maps


DEFAULT_BUILD = {"loop_order": "c_dj"}


def run(x: np.ndarray, weight: np.ndarray, bias: np.ndarray, repeat: int = 1, **kw):
    bkw = {**DEFAULT_BUILD, **kw}
    nc = build_program(float(bias[0]), repeat=repeat, **bkw)
    in_maps = _prepare_inputs(x, weight)
    res = run_bass_kernel_spmd(nc, in_maps, list(range(N_CORES)))
    full = np.concatenate([res.results[c]["y"] for c in range(N_CORES)], axis=0)
    return np.ascontiguousarray(full[:OUT_H]).astype(np.float32)


def kernel(x: np.ndarray, weight: np.ndarray, bias: np.ndarray) -> np.ndarray:
    return run(x, weight, bias, repeat=1)


# revision 20
# speedup vs baseline: 546.4283x; 1.0029x over previous
"""TRN2 Bass kernel: 4096x4096 fp32 'valid' cross-correlation with a 15x15
kernel (+ scalar bias), sharded row-wise across 8 NeuronCores.

Formulation (per core, per output row-tile of M<=114 rows):
  out[i, j] = sum_dj sum_di w[di, dj] * x[i + di, j + dj]
For each kernel column dj, the contraction over di is a banded-Toeplitz
matmul over input rows: stationary lhsT[k, m] = w[k - m, dj] (15-diagonal
band, K = M + 14 input rows), moving rhs = x rows with a free-dim column
offset of dj. The 15 dj-matmuls accumulate in one PSUM bank. Matmuls run
in float32r (TF32): 1 col/cycle on the PE at ~1e-4 relative error.

Each core gets 512 padded output rows (input slice of 526 rows including
the 14-row halo); the host pads x to 4110 rows and drops the 14 garbage
output rows at the end.
"""

import os
import sys

for _p in ("/opt/trn_rl_repo",):
    if os.path.isdir(_p) and _p not in sys.path:
        sys.path.insert(0, _p)

import json

import numpy as np

import concourse.bass as bass
import concourse.tile as tile
from concourse import mybir
from concourse.bass_utils import run_bass_kernel_spmd

# ---------------------------------------------------------------------------
# Workaround: the installed walrus_driver rejects instructions carrying more
# than one sync wait ("Too many sync wait commands"). TileContext's kernel-tail
# drain carries one wait per outstanding semaphore. Splitting each extra wait
# into its own single-wait EventSemaphore on the same engine right before the
# original instruction is semantically identical (same-engine program order;
# semaphores are monotone).
# ---------------------------------------------------------------------------
_orig_to_json_bytes = bass.Bass.to_json_bytes


def _split_multi_waits(bir: dict) -> dict:
    n = 0
    for fn in bir.get("functions", []):
        for bb in fn.get("blocks", []):
            insts = bb.get("instructions")
            if not insts:
                continue
            out = []
            for inst in insts:
                si = inst.get("sync_info") or {}
                waits = si.get("on_wait") or []
                if len(waits) > 1:
                    for w in waits[:-1]:
                        n += 1
                        out.append(
                            {
                                "debug": inst.get("debug", 0),
                                "engine": inst["engine"],
                                "ins": [],
                                "name": f"{inst['name']}-waitsplit{n}",
                                "opcode": "EventSemaphore",
                                "outs": [],
                                "sync_info": {"on_update": [], "on_wait": [w]},
                            }
                        )
                    si["on_wait"] = [waits[-1]]
                out.append(inst)
            bb["instructions"] = out
    return bir


def _patched_to_json_bytes(self, *args, **kwargs):
    raw = _orig_to_json_bytes(self, *args, **kwargs)
    return json.dumps(_split_multi_waits(json.loads(raw))).encode()


bass.Bass.to_json_bytes = _patched_to_json_bytes

# ---------------------------------------------------------------------------

H = W = 4096
KS = 15
OUT_H = H - KS + 1  # 4082
OUT_W = W - KS + 1  # 4082
N_CORES = 8
ROWS_PER_CORE = 512  # padded output rows per core (8 * 512 = 4096 >= 4082)
IN_ROWS = ROWS_PER_CORE + KS - 1  # 526

# Output row-tiles per core: M <= 114 so the band (M + 14) fits in K <= 128.
M_TILES = [(0, 114), (114, 114), (228, 114), (342, 114), (456, 56)]
# Corner scheme: 4 tiles of 128 rows; the 14 seam rows per tile are completed
# by two extra matmuls contracting (dj, k') pairs over replicated shifted
# copies of the 14 rows below the tile's K-block.
M_TILES_CORNER = [(0, 128), (128, 128), (256, 128), (384, 128)]
CORNER_SPLIT = 9  # dj 0..8 -> corner MM a (K=126), dj 9..14 -> MM b (K=84)
# Output column tiles: N <= 512 (one fp32 PSUM bank).
N_TILES = [(c, min(512, OUT_W - c)) for c in range(0, OUT_W, 512)]

F32R = mybir.dt.float32r
F32 = mybir.dt.float32


def build_program(
    bias_val: float,
    repeat: int = 1,
    loop_repeat: int = 1,
    loop_order: str = "c_dj",
    evacuate: bool = True,
    psum_bufs: int = 4,
    same_stationary: bool = False,
    pure_mm: bool = False,
    x_bufs: int = 2,
    y_per_ctile: bool = False,
    split_dma: int = 1,
    evac_any: bool = False,
) -> bass.Bass:
    nc = bass.Bass()
    x_d = nc.dram_tensor("x", [IN_ROWS, W], F32R, kind="ExternalInput")
    t_d = nc.dram_tensor("t", [128, KS, 128], F32R, kind="ExternalInput")
    t2_d = nc.dram_tensor("t2", [14 * KS, 128], F32R, kind="ExternalInput")
    r_d = nc.dram_tensor(
        "r", [len(M_TILES_CORNER), 14 * KS, OUT_W], F32R, kind="ExternalInput"
    )
    y_d = nc.dram_tensor("y", [ROWS_PER_CORE, OUT_W], F32, kind="ExternalOutput")

    with tile.TileContext(nc) as tc:
        with (
            tc.tile_pool(name="tconst", bufs=1) as tpool,
            tc.tile_pool(name="xin", bufs=x_bufs) as xpool,
            tc.tile_pool(name="rrep", bufs=2) as rpool,
            tc.tile_pool(name="yout", bufs=2) as ypool,
            tc.tile_pool(name="acc", bufs=psum_bufs, space="PSUM") as psum,
        ):
            t_s = tpool.tile([128, KS, 128], F32R)
            nc.sync.dma_start(t_s[:], t_d[:])
            ka = 14 * CORNER_SPLIT  # 126
            kb = 14 * (KS - CORNER_SPLIT)  # 84
            if loop_order == "corner":
                t2a_s = tpool.tile([ka, 128], F32R)
                t2b_s = tpool.tile([kb, 128], F32R)
                nc.sync.dma_start(t2a_s[:], t2_d[0:ka, :])
                nc.sync.dma_start(t2b_s[:], t2_d[ka : ka + kb, :])

            def mtile_c_dj(m0, m, kp, x_s, y_s):
                for c0, n in N_TILES:
                    acc = psum.tile([128, 512], F32, tag="acc")
                    for dj in range(KS):
                        nc.tensor.matmul(
                            acc[0:m, 0:n],
                            t_s[0:kp, 0 if same_stationary else dj, 0:m],
                            x_s[0:kp, c0 + dj : c0 + dj + n],
                            start=(dj == 0),
                            stop=(dj == KS - 1),
                        )
                    if evacuate:
                        eng = nc.any if evac_any else nc.vector
                        eng.tensor_scalar_add(
                            y_s[0:m, c0 : c0 + n], acc[0:m, 0:n], bias_val
                        )
                        if y_per_ctile:
                            nc.sync.dma_start(
                                y_d[m0 : m0 + m, c0 : c0 + n],
                                y_s[0:m, c0 : c0 + n],
                            )

            def mtile_dj_c(m0, m, kp, x_s, y_s):
                accs = [
                    psum.tile([128, 512], F32, tag=f"acc{i}", name=f"acc{i}")
                    for i in range(len(N_TILES))
                ]
                for dj in range(KS):
                    for ci, (c0, n) in enumerate(N_TILES):
                        nc.tensor.matmul(
                            accs[ci][0:m, 0:n],
                            t_s[0:kp, dj, 0:m],
                            x_s[0:kp, c0 + dj : c0 + dj + n],
                            start=(dj == 0),
                            stop=(dj == KS - 1),
                        )
                if evacuate:
                    for ci, (c0, n) in enumerate(N_TILES):
                        nc.vector.tensor_scalar_add(
                            y_s[0:m, c0 : c0 + n], accs[ci][0:m, 0:n], bias_val
                        )

            def mtile_corner(m0, x_s, ra, rb, y_s):
                for c0, n in N_TILES:
                    acc = psum.tile([128, 512], F32, tag="acc")
                    for dj in range(KS):
                        nc.tensor.matmul(
                            acc[:, 0:n],
                            t_s[:, dj, :],
                            x_s[:, c0 + dj : c0 + dj + n],
                            start=(dj == 0),
                            stop=False,
                        )
                    nc.tensor.matmul(
                        acc[:, 0:n],
                        t2a_s[:],
                        ra[0:ka, c0 : c0 + n],
                        start=False,
                        stop=False,
                    )
                    nc.tensor.matmul(
                        acc[:, 0:n],
                        t2b_s[:],
                        rb[0:kb, c0 : c0 + n],
                        start=False,
                        stop=True,
                    )
                    if evacuate:
                        nc.vector.tensor_scalar_add(
                            y_s[:, c0 : c0 + n], acc[:, 0:n], bias_val
                        )

            def body_corner():
                for _ in range(repeat):
                    for ti, (m0, m) in enumerate(M_TILES_CORNER):
                        x_s = xpool.tile([128, W], F32R, tag="xs")
                        nc.sync.dma_start(x_s[:], x_d[m0 : m0 + 128, :])
                        ra = rpool.tile([128, OUT_W], F32R, tag="ra")
                        rb = rpool.tile([128, OUT_W], F32R, tag="rb")
                        nc.sync.dma_start(ra[0:ka, :], r_d[ti, 0:ka, :])
                        nc.sync.dma_start(rb[0:kb, :], r_d[ti, ka : ka + kb, :])
                        y_s = ypool.tile([128, OUT_W], F32, tag="ys")
                        mtile_corner(m0, x_s, ra, rb, y_s)
                        if evacuate:
                            nc.sync.dma_start(y_d[m0 : m0 + 128, :], y_s[:])

            def body():
                if loop_order == "corner":
                    body_corner()
                    return
                for _ in range(repeat):
                    for m0, m in M_TILES:
                        kp = m + KS - 1
                        x_s = xpool.tile([128, W], F32R, tag="xs")
                        if split_dma > 1:
                            step = (kp + split_dma - 1) // split_dma
                            for p in range(0, kp, step):
                                pe = min(p + step, kp)
                                nc.sync.dma_start(
                                    x_s[p:pe, :], x_d[m0 + p : m0 + pe, :]
                                )
                        else:
                            nc.sync.dma_start(x_s[0:kp, :], x_d[m0 : m0 + kp, :])
                        y_s = ypool.tile([128, OUT_W], F32, tag="ys")
                        if loop_order == "c_dj":
                            mtile_c_dj(m0, m, kp, x_s, y_s)
                        else:
                            mtile_dj_c(m0, m, kp, x_s, y_s)
                        if evacuate and not (loop_order == "c_dj" and y_per_ctile):
                            if split_dma > 1:
                                cstep = (OUT_W + split_dma - 1) // split_dma
                                for c in range(0, OUT_W, cstep):
                                    ce = min(c + cstep, OUT_W)
                                    nc.sync.dma_start(
                                        y_d[m0 : m0 + m, c:ce], y_s[0:m, c:ce]
                                    )
                            else:
                                nc.sync.dma_start(y_d[m0 : m0 + m, :], y_s[0:m, :])

            def body_pure_mm():
                x_s = xpool.tile([128, W], F32R, tag="xs")
                nc.sync.dma_start(x_s[:], x_d[0:128, :])

                def inner():
                    for _ in range(repeat):
                        for m0, m in M_TILES:
                            kp = m + KS - 1
                            for c0, n in N_TILES:
                                acc = psum.tile([128, 512], F32, tag="acc")
                                for dj in range(KS):
                                    nc.tensor.matmul(
                                        acc[0:m, 0:n],
                                        t_s[0:kp, dj, 0:m],
                                        x_s[0:kp, c0 + dj : c0 + dj + n],
                                        start=(dj == 0),
                                        stop=(dj == KS - 1),
                                    )

                if loop_repeat > 1:
                    with tc.For_i(0, loop_repeat, 1):
                        inner()
                else:
                    inner()

            if pure_mm:
                body_pure_mm()
            elif loop_repeat > 1:
                with tc.For_i(0, loop_repeat, 1):
                    body()
            else:
                body()
    return nc


def _toeplitz(weight: np.ndarray) -> np.ndarray:
    """T[k, dj, m] = weight[k - m, dj] for 0 <= k - m < 15, else 0."""
    t = np.zeros((128, KS, 128), dtype=np.float32)
    k = np.arange(128)[:, None]
    m = np.arange(128)[None, :]
    d = k - m  # [128, 128]
    mask = (d >= 0) & (d < KS)
    for dj in range(KS):
        col = np.zeros((128, 128), dtype=np.float32)
        col[mask] = weight[d[mask], dj]
        t[:, dj, :] = col
    return t


def _toeplitz_corner(weight: np.ndarray) -> np.ndarray:
    """T2[14*dj + k', m] = weight[128 + k' - m, dj] for m in [114+k', 127]."""
    t2 = np.zeros((14 * KS, 128), dtype=np.float32)
    for dj in range(KS):
        for k_ in range(14):
            m = np.arange(114 + k_, 128)
            t2[14 * dj + k_, m] = weight[128 + k_ - m, dj]
    return t2


def _replicated_seam(x_core: np.ndarray) -> np.ndarray:
    """r[tile, 14*dj' + k', q] = x_core[128*tile + 128 + k', q + dj], with the
    dj >= CORNER_SPLIT blocks packed after the first 14*CORNER_SPLIT rows."""
    r = np.zeros((len(M_TILES_CORNER), 14 * KS, OUT_W), dtype=np.float32)
    for ti, (m0, _) in enumerate(M_TILES_CORNER):
        rows = x_core[m0 + 128 : m0 + 142]  # [14, W]
        for dj in range(KS):
            p0 = (
                14 * dj
                if dj < CORNER_SPLIT
                else 14 * CORNER_SPLIT + 14 * (dj - CORNER_SPLIT)
            )
            r[ti, p0 : p0 + 14, :] = rows[:, dj : dj + OUT_W]
    return r


def _prepare_inputs(x: np.ndarray, weight: np.ndarray):
    x_pad = np.zeros((N_CORES * ROWS_PER_CORE + KS - 1, W), dtype=np.float32)
    x_pad[:H] = x
    t = _toeplitz(weight)
    t2 = _toeplitz_corner(weight)
    in_maps = []
    for c in range(N_CORES):
        r0 = c * ROWS_PER_CORE
        xc = np.ascontiguousarray(x_pad[r0 : r0 + IN_ROWS])
        in_maps.append(
            {"x": xc, "t": t, "t2": t2, "r": _replicated_seam(xc)}
        )
    return in_maps


DEFAULT_BUILD = {"loop_order": "corner", "psum_bufs": 6}


def run(x: np.ndarray, weight: np.ndarray, bias: np.ndarray, repeat: int = 1, **kw):
    bkw = {**DEFAULT_BUILD, **kw}
    nc = build_program(float(bias[0]), repeat=repeat, **bkw)
    in_maps = _prepare_inputs(x, weight)
    res = run_bass_kernel_spmd(nc, in_maps, list(range(N_CORES)))
    full = np.concatenate([res.results[c]["y"] for c in range(N_CORES)], axis=0)
    return np.ascontiguousarray(full[:OUT_H]).astype(np.float32)


def kernel(x: np.ndarray, weight: np.ndarray, bias: np.ndarray) -> np.ndarray:
    return run(x, weight, bias, repeat=1)
